# revision 33
# baseline (speedup 1.0000x reference)
"""Trainium2 Bass kernel for nn_BatchedSpGat (2-layer GAT + L2-normalize + relu).

Strategy (8 NeuronCores, SPMD single program):
  - Nodes sharded contiguously: core c owns nodes [c*NPC, (c+1)*NPC).
    Tables are padded to NPCP = DT*128 rows per core so every tile DMA is a
    uniform 128-row block; gather indices address physical (padded) rows.
  - Edges assigned to the owner of their DST node, grouped by (dst-tile-of-128,
    src-half), padded so every (dst-tile, half) group is a fixed number of
    128-edge tiles (uniform across cores -> one SPMD program). The lo/hi halves
    (phys row < 32768 vs >=, for int16 gather indices) of a chunk land in one
    combined SBUF tile so all elementwise work runs as one fused op per chunk.
  - Layer 1: sharded GEMM (own nodes, bf16) -> AllGather of a per-node bf16
    table [h1 | ee-slot | al_src(f32 bits) | pad] -> per-edge dma_gather of
    table rows + small gather of al_dst -> exp(leaky(al_s+al_d)) written into
    the gathered rows' ee-slot -> ONE one-hot matmul per 128-edge tile
    accumulates softmax numerator AND denominator in PSUM -> normalize.
  - Layer 2: AllGather the transposed layer-1 output (bf16) and let every core
    redundantly compute GEMM2 for ALL nodes, building the layer-2 gather table
    [h2 | 1.0 | al_src(f32 bits) | pad] locally (no second table AllGather).
    The constant-1 column folds the denominator into the one-hot matmul.
  - Softmax uses no max-subtraction (logits empirically bounded ~14; exp is
    safe in fp32; alpha is shift-invariant so the result is identical).

kernel(**inputs) takes the FULL problem inputs and returns the FULL output.
Repeat calls reuse the compiled program and device-staged inputs (inputs are
content-hashed; any change re-stages them).
"""
import os
import sys
import zlib
from contextlib import ExitStack

import numpy as np

for _p in ('/opt/trn_rl_repo', '/root/.axon_site/_ro/trn_rl_repo'):
    if os.path.isdir(_p) and _p not in sys.path:
        sys.path.insert(0, _p)

import concourse.bass as bass
import concourse.bacc as bacc
import concourse.tile as tile
import concourse.mybir as mybir
from concourse.bass import AP
from concourse.library_config import mlp as _mlp_lib
from concourse.masks import make_identity

F32 = mybir.dt.float32
BF16 = mybir.dt.bfloat16
I16 = mybir.dt.int16
OP = mybir.AluOpType
AF = mybir.ActivationFunctionType

NEG_SLOPE = 0.2

NP_BF16 = mybir.dt.np(BF16)


class Cfg:
    def __init__(self, N=50000, E=800000, cores=8, half=32768,
                 F0=512, F1=128, H1=4, F2=256, CD=2, G=7):
        self.N = N                  # nodes
        self.E = E                  # edges (before self-loops)
        self.CORES = cores
        self.HALF = half            # src-half split for int16 gather idx
        self.F0 = F0                # input features
        self.F1 = F1                # layer-1 out features (H1 * C1)
        self.H1 = H1                # layer-1 heads
        self.C1 = F1 // H1
        self.F2 = F2                # layer-2 out features (1 head)
        self.CD = CD                # dst-tiles per aggregation chunk
        self.G = G                  # dst-tiles per GEMM group
        assert N % cores == 0
        self.NPC = N // cores       # nodes per core
        self.DT = (self.NPC + 127) // 128   # dst tiles per core
        self.NPCP = self.DT * 128   # padded rows per core
        self.KT = F0 // 128         # k-tiles for GEMM1
        # bf16 table row widths (gather rows/strides must be 256B multiples,
        # i.e. multiples of 128 bf16 elements)
        # table1 row: [h1(128) | ee-slot(4) | als1 f32 bits(8) | pad] -> 256
        self.ST1 = 256
        self.EE1 = F1               # ee slot offset (cols 128:132)
        self.AS1 = F1 + H1          # als1 f32-bits offset (cols 132:140)
        # table2 row: [h2(256)] — als2 is recomputed from the gathered row,
        # the softmax denominator comes from a ones-rhs matmul
        self.ST2 = 256
        self.ALS = 64               # al_own row width in f32 (ald | pad)


# ---------------------------------------------------------------------------
# Host-side preprocessing
# ---------------------------------------------------------------------------

def preprocess(edge_index, cfg: Cfg):
    """Partition/pad edges. Returns (percore, T_LO, T_HI).

    percore[c] holds:
      gidx_{lo,hi}  int16 [128, NS_S*8]  wrapped gather indices, slot-major
                    (slot = tile*T_S + j), idx = physical padded row - base
      didx_{lo,hi}  int16 [128, NS_S*8]  wrapped local-dst indices
      dstrel        f32   [128, NS_ALL]  dst_local - tile*128 (-1 dummies),
                    CHUNK-MAJOR combined layout: per chunk of CD tiles,
                    [lo slots (tile-major) | hi slots (tile-major)]
    """
    N, NPC, NPCP, DT, HALF, CORES, CD = (cfg.N, cfg.NPC, cfg.NPCP, cfg.DT,
                                         cfg.HALF, cfg.CORES, cfg.CD)
    src = np.concatenate([np.asarray(edge_index[0], np.int64),
                          np.arange(N, dtype=np.int64)])
    dst = np.concatenate([np.asarray(edge_index[1], np.int64),
                          np.arange(N, dtype=np.int64)])
    # physical padded row of each source node
    sphys = (src // NPC) * NPCP + (src % NPC)
    owner = dst // NPC

    groups = []
    for c in range(CORES):
        m = owner == c
        s_c, d_c = sphys[m], dst[m]
        dl = d_c - c * NPC
        dt = dl // 128
        order = np.argsort(dt, kind='stable')
        s_c, dl_c, dt_c = s_c[order], dl[order], dt[order]
        lo = s_c < HALF
        bounds = np.searchsorted(dt_c, np.arange(DT + 1))
        groups.append((s_c, dl_c, lo, bounds))

    def tiles_needed(c, t, want_lo):
        s_c, dl_c, lo, bounds = groups[c]
        sl = slice(bounds[t], bounds[t + 1])
        k = int(np.count_nonzero(lo[sl] == want_lo))
        return (k + 127) // 128

    T_LO = max(1, max(tiles_needed(c, t, True)
                      for c in range(CORES) for t in range(DT)))
    T_HI = max(1, max(tiles_needed(c, t, False)
                      for c in range(CORES) for t in range(DT)))
    T_ALL = T_LO + T_HI

    percore = []
    for c in range(CORES):
        s_c, dl_c, lo, bounds = groups[c]
        arrs = {}
        pert = {}   # (tag, t) -> (didx_vals, drel_vals) padded per tile
        for tag, want_lo, T_S in (('lo', True, T_LO), ('hi', False, T_HI)):
            nslot = DT * T_S
            tot = nslot * 128
            gidx = np.zeros(tot, np.int16)
            for t in range(DT):
                sl = slice(bounds[t], bounds[t + 1])
                m = lo[sl] == want_lo
                s_t = s_c[sl][m]
                dl_t = dl_c[sl][m]
                k = len(s_t)
                o = t * T_S * 128
                gidx[o:o + k] = (s_t - (0 if want_lo else HALF)).astype(np.int16)
                di = np.zeros(T_S * 128, np.int16)
                di[:k] = dl_t.astype(np.int16)
                dr = np.full(T_S * 128, -1.0, np.float32)
                dr[:k] = (dl_t - t * 128).astype(np.float32)
                pert[(tag, t)] = (di, dr)
            w16 = gidx.reshape(-1, 16).T                      # [16, tot/16]
            arrs['gidx_' + tag] = np.ascontiguousarray(np.tile(w16, (8, 1)))
        # combined chunk-major dstrel (bf16) and didx
        drel_cmb = np.empty((DT * T_ALL, 128), np.float32)
        didx_cmb = np.empty(DT * T_ALL * 128, np.int16)
        pos = 0
        for t0 in range(0, DT, CD):
            nd = min(CD, DT - t0)
            for tag, T_S in (('lo', T_LO), ('hi', T_HI)):
                for t in range(t0, t0 + nd):
                    di, dr = pert[(tag, t)]
                    drel_cmb[pos:pos + T_S] = dr.reshape(T_S, 128)
                    didx_cmb[pos * 128:(pos + T_S) * 128] = di
                    pos += T_S
        assert pos == DT * T_ALL
        arrs['dstrel'] = np.ascontiguousarray(drel_cmb.T.astype(NP_BF16))
        d16 = didx_cmb.reshape(-1, 16).T
        arrs['didx'] = np.ascontiguousarray(np.tile(d16, (8, 1)))
        percore.append(arrs)
    return percore, T_LO, T_HI


def make_in_maps(inputs, cfg: Cfg, percore, T_LO, T_HI, xT_b16=None):
    N, NPC, F0, F1, H1, F2 = cfg.N, cfg.NPC, cfg.F0, cfg.F1, cfg.H1, cfg.F2
    if xT_b16 is None:
        x = np.asarray(inputs['x'], np.float32).reshape(N, F0)
        xT_b16 = np.ascontiguousarray(x.T.astype(NP_BF16))    # [F0, N]
    W1 = np.asarray(inputs['W1'], np.float32)
    W2 = np.asarray(inputs['W2'], np.float32)
    a1s = np.asarray(inputs['a1_s'], np.float32)              # [H1, C1]
    a1d = np.asarray(inputs['a1_d'], np.float32)
    a2s = np.asarray(inputs['a2_s'], np.float32).reshape(F2, 1)
    a2d = np.asarray(inputs['a2_d'], np.float32).reshape(F2, 1)
    b1 = np.asarray(inputs['b1'], np.float32).reshape(1, F1)
    b2 = np.asarray(inputs['b2'], np.float32).reshape(1, F2)

    # fold the attention projections into extra GEMM output columns:
    # W1_ext = [W1 | W1 @ As | W1 @ Ad]  (block-diagonal per head)
    C1 = F1 // H1
    As = np.zeros((F1, H1), np.float32)
    Ad = np.zeros((F1, H1), np.float32)
    for h in range(H1):
        As[h * C1:(h + 1) * C1, h] = a1s[h]
        Ad[h * C1:(h + 1) * C1, h] = a1d[h]
    W1_ext = np.concatenate([W1, W1 @ As, W1 @ Ad], axis=1)   # [F0, F1+2H]
    W2_ext = np.concatenate([W2, W2 @ a2s, W2 @ a2d], axis=1)  # [F1, F2+2]

    shared = {
        'W1': np.ascontiguousarray(W1_ext.astype(NP_BF16)),
        'W2': np.ascontiguousarray(W2_ext.astype(NP_BF16)),
        'b1_rep': np.ascontiguousarray(np.tile(b1, (128, 1))),
        'b2_rep': np.ascontiguousarray(np.tile(b2, (128, 1))),
        'a2s_repb': np.ascontiguousarray(
            np.tile(a2s.reshape(1, F2), (128, 1)).astype(NP_BF16)),
        'iota128': np.ascontiguousarray(
            np.tile(np.arange(128, dtype=np.float32), (128, 1))
            .astype(NP_BF16)),
    }
    in_maps = []
    for c in range(cfg.CORES):
        m = dict(shared)
        m['xT'] = np.ascontiguousarray(xT_b16[:, c * NPC:(c + 1) * NPC])
        m.update(percore[c])
        in_maps.append(m)
    return in_maps


# ---------------------------------------------------------------------------
# Device program
# ---------------------------------------------------------------------------

def _mid_bcast(ap2d: AP, T: int) -> AP:
    """[128, W] -> [128, T(stride 0), W] view."""
    return AP(ap2d.tensor, ap2d.offset, [ap2d.ap[0], [0, T], ap2d.ap[1]])


def _rows(dram, c0, nt, width0, width1):
    """[nt*128 rows, width] DRAM slice viewed as [128, nt, width]."""
    return dram[c0:c0 + nt * 128, width0:width1].rearrange(
        '(t p) c -> p t c', p=128)


def build_program(cfg: Cfg, T_LO, T_HI, stop='full'):
    c = cfg
    DT, NPC, NPCP, F0, F1, H1, F2, ST1, ST2, KT, G = (
        c.DT, c.NPC, c.NPCP, c.F0, c.F1, c.H1, c.F2, c.ST1, c.ST2, c.KT, c.G)
    NS_LO, NS_HI = DT * T_LO, DT * T_HI
    T_ALL = T_LO + T_HI
    NS_ALL = DT * T_ALL
    CORES = c.CORES

    nc = bacc.Bacc('TRN2', target_bir_lowering=False, debug=False,
                   num_devices=CORES, num_swdge_queues=4)

    # --- I/O -------------------------------------------------------------
    W1X = F1 + 2 * H1           # GEMM1 cols: h1 | als1 | ald1
    W2X = F2 + 2                # GEMM2 cols: h2 | als2 | ald2
    d_xT = nc.dram_tensor('xT', [F0, NPC], BF16, kind='ExternalInput')
    d_W1 = nc.dram_tensor('W1', [F0, W1X], BF16, kind='ExternalInput')
    d_W2 = nc.dram_tensor('W2', [F1, W2X], BF16, kind='ExternalInput')
    d_reps = {}
    for nm, w, dt_ in (('b1_rep', F1, F32), ('b2_rep', F2, F32),
                       ('a2s_repb', F2, BF16), ('iota128', 128, BF16)):
        d_reps[nm] = nc.dram_tensor(nm, [128, w], dt_, kind='ExternalInput')
    d_idx = {}
    for tag, ns in (('lo', NS_LO), ('hi', NS_HI)):
        d_idx['gidx_' + tag] = nc.dram_tensor(
            'gidx_' + tag, [128, ns * 8], I16, kind='ExternalInput')
    d_idx['didx'] = nc.dram_tensor(
        'didx', [128, NS_ALL * 8], I16, kind='ExternalInput')
    d_idx['dstrel'] = nc.dram_tensor(
        'dstrel', [128, NS_ALL], BF16, kind='ExternalInput')
    d_out = nc.dram_tensor('out', [NPC, F2], F32, kind='ExternalOutput')

    # internal DRAM (padded rows)
    t1own = nc.dram_tensor('t1own', [NPCP, ST1], BF16, kind='Internal')
    al1own = nc.dram_tensor('al1own', [NPCP, c.ALS], F32, kind='Internal')
    al2own = nc.dram_tensor('al2own', [NPCP, c.ALS], F32, kind='Internal')
    table1 = nc.dram_tensor('table1', [NPCP * CORES, ST1], BF16,
                            kind='Internal', addr_space='Shared')
    # h1^T is AllGathered in G-tile column chunks so the collectives pipeline
    # behind the layer-1 aggregation that produces them
    n_agc = (DT + G - 1) // G
    h1town_k, h1T_k = [], []
    for k in range(n_agc):
        gw = min(G, DT - k * G) * 128
        h1town_k.append(nc.dram_tensor(f'h1town{k}', [128, gw], BF16,
                                       kind='Internal'))
        h1T_k.append(nc.dram_tensor(f'h1T{k}', [128 * CORES, gw], BF16,
                                    kind='Internal', addr_space='Shared'))
    h2full = nc.dram_tensor('h2full', [NPCP * CORES, ST2], BF16,
                            kind='Internal')

    rg = [list(range(CORES))]
    NROWS = NPCP * CORES

    def _body(tc, S):
        nc.gpsimd.load_library(_mlp_lib)
        P = S.enter_context(tc.tile_pool(name='persist', bufs=1))

        # persistent SBUF constants / index arrays
        sb = {}
        W1sb = P.tile([128, KT, W1X], BF16, tag='W1sb')
        for k in range(KT):
            nc.sync.dma_start(W1sb[:, k, :], d_W1[k * 128:(k + 1) * 128, :])
        W2sb = P.tile([128, W2X], BF16, tag='W2sb')
        nc.sync.dma_start(W2sb[:], d_W2[:, :])
        for nm, w, dt_ in (('b1_rep', F1, F32), ('b2_rep', F2, F32),
                           ('a2s_repb', F2, BF16), ('iota128', 128, BF16)):
            sb[nm] = P.tile([128, w], dt_, tag=nm, name=nm)
            nc.sync.dma_start(sb[nm][:], d_reps[nm][:, :])
        for nm, ns in (('gidx_lo', NS_LO), ('gidx_hi', NS_HI),
                       ('didx', NS_ALL)):
            sb[nm] = P.tile([128, ns * 8], I16, tag=nm, name=nm)
            nc.sync.dma_start(sb[nm][:], d_idx[nm][:, :])
        sb['dstrel'] = P.tile([128, NS_ALL], BF16, tag='dstrel',
                              name='dstrel')
        nc.sync.dma_start(sb['dstrel'][:], d_idx['dstrel'][:, :])
        ident = P.tile([128, 128], BF16, tag='ident')
        make_identity(nc, ident[:])
        ones = P.tile([128, 1], BF16, tag='ones')
        nc.vector.memset(ones[:], 1.0)

        # ---------------- Phase 1: GEMM1 + table1 rows -------------------
        with ExitStack() as S1:
            xp = S1.enter_context(tc.tile_pool(name='xslab', bufs=1))
            p1 = S1.enter_context(tc.tile_pool(name='p1sb', bufs=3))
            pp1 = S1.enter_context(
                tc.tile_pool(name='p1ps', bufs=4, space='PSUM'))
            xTsb = xp.tile([128, KT, NPC], BF16)
            for k in range(KT):
                nc.sync.dma_start(xTsb[:, k, :],
                                  d_xT[k * 128:(k + 1) * 128, :])
            for g0 in range(0, DT, G):
                ng = min(G, DT - g0)
                slabb = p1.tile([128, G, F1], BF16, tag='slabb')
                alss = p1.tile([128, G, 2 * H1], F32, tag='alss')
                for t in range(ng):
                    m = g0 + t
                    c0 = m * 128
                    ph = min(128, NPC - c0)
                    ps = pp1.tile([128, W1X], F32, space='PSUM')
                    for k in range(KT):
                        nc.tensor.matmul(ps[:ph, :],
                                         lhsT=xTsb[:, k, c0:c0 + ph],
                                         rhs=W1sb[:, k, :],
                                         start=(k == 0), stop=(k == KT - 1))
                    nc.scalar.copy(slabb[:, t, :], ps[:, 0:F1])
                    nc.scalar.copy(alss[:, t, :], ps[:, F1:F1 + 2 * H1])
                nc.scalar.dma_start(_rows(t1own, g0 * 128, ng, 0, F1),
                                    slabb[:, 0:ng, :])
                nc.scalar.dma_start(
                    _rows(t1own, g0 * 128, ng, c.AS1, c.AS1 + 2 * H1),
                    alss[:, 0:ng, 0:H1].bitcast(BF16))
                nc.scalar.dma_start(_rows(al1own, g0 * 128, ng, 0, H1),
                                    alss[:, 0:ng, H1:2 * H1])

        def _dbg_out(src_dram, rows, width, dtype=F32):
            dp = tc.tile_pool(name='dbg', bufs=1)
            with dp as dpp:
                for r0 in range(0, rows, 128):
                    pr = min(128, rows - r0)
                    t_ = dpp.tile([128, width], dtype, tag='dbgt', name='dbgt')
                    nc.sync.dma_start(t_[:pr, :], src_dram[r0:r0 + pr, 0:width])
                    o_ = dpp.tile([128, width], F32, tag='dbgo', name='dbgo')
                    nc.vector.tensor_copy(o_[:pr, :], t_[:pr, :])
                    nc.sync.dma_start(
                        d_out[r0:r0 + pr, 0:min(width, F2)],
                        o_[:pr, 0:min(width, F2)])

        if stop == 'p1':
            _dbg_out(t1own, NPC, min(ST1, F2), BF16)
            return

        # ---------------- Phase 2: AllGather table1 ----------------------
        if CORES == 1:
            nc.sync.dma_start(table1[:, :], t1own[:, :])
        else:
            nc.gpsimd.collective_compute(
                'AllGather', OP.bypass, replica_groups=rg,
                ins=[t1own[:, :]], outs=[table1[:, :]])
        if stop == 'ag1':
            _dbg_out(table1[NPCP:NPCP + NPC, :], NPC, min(ST1, F2), BF16)
            return

        # ---------------- Aggregation (shared by both layers) ------------
        def aggregate(table, al_own, ST, F, H, layer, emit):
            """Per-edge gather + one-hot-matmul segment softmax.

            emit(t0, nd, Us, fp) is called per chunk with the list of PSUM
            tiles per dst tile: layer 1 -> U (numerator|denominator cols),
            layer 2 -> (U, sU) (numerator, denominator).
            """
            CDn = c.CD
            RW = (F + H) if layer == 1 else F
            with ExitStack() as SA:
                gp = SA.enter_context(tc.tile_pool(
                    name=f'g{layer}', bufs=3))
                cp = SA.enter_context(tc.tile_pool(
                    name=f'c{layer}', bufs=2))
                sp = SA.enter_context(tc.tile_pool(
                    name=f's{layer}', bufs=2))
                up = SA.enter_context(tc.tile_pool(
                    name=f'u{layer}', bufs=4, space='PSUM'))
                if layer == 2:
                    spA = SA.enter_context(tc.tile_pool(
                        name=f'sA{layer}', bufs=1))
                    up2 = SA.enter_context(tc.tile_pool(
                        name=f'u2{layer}', bufs=4, space='PSUM'))
                fp = SA.enter_context(tc.tile_pool(name=f'f{layer}', bufs=3))

                for t0 in range(0, DT, CDn):
                    nd = min(CDn, DT - t0)
                    cd_lo, cd_hi = nd * T_LO, nd * T_HI
                    cd = cd_lo + cd_hi
                    a_cmb = t0 * T_ALL
                    Hc = gp.tile([128, CDn * T_ALL, ST], BF16, tag='Hc')
                    aldt = sp.tile([128, CDn * T_ALL, 64], F32, tag='ald')
                    # byte-balanced gather spread over the 4 SWDGE queues:
                    # q0: first part of lo, q1: all of hi,
                    # q3: rest of lo + first part of didx, q2: rest of didx
                    hc_u = ST // 128          # 32KB units per table slot
                    per_q = (cd * hc_u + cd) / 4.0
                    s0 = min(cd_lo, max(1, int(round(per_q / hc_u))))
                    d0 = min(cd, max(0, int(round(
                        per_q - (cd_lo - s0) * hc_u))))
                    for o0, cds, base, nrows, gi, a, q in (
                            (0, s0, 0, c.HALF,
                             'gidx_lo', t0 * T_LO, 0),
                            (s0, cd_lo - s0, 0, c.HALF,
                             'gidx_lo', t0 * T_LO + s0, 3),
                            (cd_lo, cd_hi, c.HALF, NROWS - c.HALF,
                             'gidx_hi', t0 * T_HI, 1)):
                        if cds == 0:
                            continue
                        ni = cds * 128
                        nc.gpsimd.dma_gather(
                            Hc[:, o0:o0 + cds, :],
                            table[base:base + nrows, 0:ST],
                            sb[gi][:, a * 8:(a + cds) * 8],
                            ni, ni, ST, elem_step=ST, single_packet=False,
                            queue_num=q)
                    for o0, cds, q in ((0, d0, 3), (d0, cd - d0, 2)):
                        if cds == 0:
                            continue
                        nc.gpsimd.dma_gather(
                            aldt[:, o0:o0 + cds, :], al_own[:, :],
                            sb['didx'][:, (a_cmb + o0) * 8:
                                       (a_cmb + o0 + cds) * 8],
                            cds * 128, cds * 128, 64, elem_step=64,
                            single_packet=False, queue_num=q)
                    # logits = als + ald ; ee = exp(leaky)
                    lsum = sp.tile([128, CDn * T_ALL, H], F32, tag='ls')
                    if layer == 1:
                        # als1 travels as raw f32 bits inside the bf16 row
                        als_v = Hc[:, 0:cd, c.AS1:c.AS1 + 2 * H].bitcast(F32)
                        nc.vector.tensor_tensor(lsum[:, 0:cd, :], als_v,
                                                aldt[:, 0:cd, 0:H], op=OP.add)
                    else:
                        # als2 recomputed from the gathered h2 row
                        sA = spA.tile([128, CDn * T_ALL, F], BF16, tag='sA')
                        nc.vector.tensor_tensor(
                            sA[:, 0:cd, :], Hc[:, 0:cd, 0:F],
                            _mid_bcast(sb['a2s_repb'][:, :], cd), op=OP.mult)
                        alsF = sp.tile([128, CDn * T_ALL, 1], F32, tag='alf')
                        nc.vector.tensor_reduce(alsF[:, 0:cd, :],
                                                sA[:, 0:cd, :],
                                                axis=mybir.AxisListType.X,
                                                op=OP.add)
                        nc.vector.tensor_tensor(lsum[:, 0:cd, :],
                                                alsF[:, 0:cd, :],
                                                aldt[:, 0:cd, 0:H], op=OP.add)
                    lk = sp.tile([128, CDn * T_ALL, H], F32, tag='lk')
                    nc.vector.scalar_tensor_tensor(
                        lk[:, 0:cd, :], lsum[:, 0:cd, :], NEG_SLOPE,
                        lsum[:, 0:cd, :], op0=OP.mult, op1=OP.max)
                    eeb = sp.tile([128, CDn * T_ALL, H], BF16, tag='eb')
                    nc.scalar.activation(eeb[:, 0:cd, :], lk[:, 0:cd, :],
                                         AF.Exp)
                    cmp = cp.tile([128, CDn * T_ALL, 128], BF16, tag='cmp')
                    drel_v = sb['dstrel'][:, a_cmb:a_cmb + cd] \
                        .to_broadcast([128, cd, 128])
                    iota_v = _mid_bcast(sb['iota128'][:, :], cd)
                    nc.vector.tensor_tensor(cmp[:, 0:cd, :], drel_v, iota_v,
                                            op=OP.is_equal)
                    if layer == 1:
                        # scale gathered h by ee per head; stash ee in the
                        # row's ee-slot so one matmul yields numerator AND
                        # denominator
                        Hv = Hc[:, 0:cd, 0:F].rearrange(
                            'p t (h cc) -> p t h cc', h=H)
                        nc.vector.tensor_tensor(
                            Hv, Hv, eeb[:, 0:cd, :].to_broadcast(
                                [128, cd, H, F // H]), op=OP.mult)
                        nc.vector.tensor_copy(Hc[:, 0:cd, c.EE1:c.EE1 + H],
                                              eeb[:, 0:cd, :])
                    else:
                        # fold ee into the one-hot lhsT; denominator comes
                        # from a second (ones-rhs) matmul
                        nc.vector.tensor_tensor(
                            cmp[:, 0:cd, :], cmp[:, 0:cd, :],
                            eeb[:, 0:cd, :].rearrange('p t h -> p (t h)')
                            .to_broadcast([128, cd, 128]), op=OP.mult)

                    Us = []
                    for ti in range(nd):
                        U = up.tile([128, RW], F32, space='PSUM')
                        sU = None
                        if layer == 2:
                            sU = up2.tile([128, 1], F32, space='PSUM',
                                          name='sU')
                        mm_i = 0
                        n_mm = T_ALL
                        for blk, T_S in ((0, T_LO), (cd_lo, T_HI)):
                            for j in range(T_S):
                                jj = blk + ti * T_S + j
                                st, sp_ = mm_i == 0, mm_i == n_mm - 1
                                nc.tensor.matmul(
                                    U[:, :], lhsT=cmp[:, jj, :],
                                    rhs=Hc[:, jj, 0:RW], start=st, stop=sp_)
                                if layer == 2:
                                    nc.tensor.matmul(
                                        sU[:, :], lhsT=cmp[:, jj, :],
                                        rhs=ones[:, :], start=st, stop=sp_)
                                mm_i += 1
                        Us.append(U if layer == 1 else (U, sU))
                    emit(t0, nd, Us, fp)

        # ---------------- Phase 3: layer-1 aggregation -------------------
        CDn = c.CD

        def emit1(t0, nd, Us, fp):
            Ub = fp.tile([128, CDn, F1 + H1], F32, tag='Ub')
            for i, U in enumerate(Us):
                nc.scalar.copy(Ub[:, i, :], U[:, :])
            s_t = fp.tile([128, CDn, H1], F32, tag='s')
            nc.vector.tensor_scalar(s_t[:, 0:nd, :],
                                    Ub[:, 0:nd, F1:F1 + H1], 1e-30, None,
                                    op0=OP.max)
            rec = fp.tile([128, CDn, H1], F32, tag='rec')
            nc.vector.reciprocal(rec[:, 0:nd, :], s_t[:, 0:nd, :])
            hL = fp.tile([128, CDn, F1], F32, tag='hL')
            nc.vector.tensor_tensor(
                hL[:, 0:nd, :].rearrange('p g (h cc) -> p g h cc', h=H1),
                Ub[:, 0:nd, 0:F1].rearrange('p g (h cc) -> p g h cc', h=H1),
                rec[:, 0:nd, :].to_broadcast([128, nd, H1, F1 // H1]),
                op=OP.mult)
            nc.vector.tensor_tensor(hL[:, 0:nd, :], hL[:, 0:nd, :],
                                    _mid_bcast(sb['b1_rep'][:, :], nd),
                                    op=OP.add)
            hLb = fp.tile([128, CDn, F1], BF16, tag='hLb')
            nc.vector.tensor_copy(hLb[:, 0:nd, :], hL[:, 0:nd, :])
            pt = ptp.tile([128, CDn * 128], BF16, space='PSUM')
            for i in range(nd):
                nc.tensor.transpose(pt[:, i * 128:(i + 1) * 128],
                                    hLb[:, i, :], ident[:, :])
            nc.vector.tensor_copy(
                h1LT[:, t0 * 128:(t0 + nd) * 128], pt[:, 0:nd * 128])
            for tt in range(t0, t0 + nd):
                k, off = tt // G, (tt % G) * 128
                nc.scalar.dma_start(h1town_k[k][:, off:off + 128],
                                    h1LT[:, tt * 128:(tt + 1) * 128])
                if tt == min(DT, (k + 1) * G) - 1:
                    # this AG chunk is complete: fire its collective so it
                    # overlaps the remaining aggregation work
                    if CORES == 1:
                        nc.sync.dma_start(h1T_k[k][0:128, :],
                                          h1town_k[k][:, :])
                    else:
                        nc.gpsimd.collective_compute(
                            'AllGather', OP.bypass, replica_groups=rg,
                            ins=[h1town_k[k][:, :]], outs=[h1T_k[k][:, :]])

        h1lt_cm = tc.tile_pool(name='h1lt', bufs=1)
        h1lt_pool = h1lt_cm.__enter__()
        h1LT = h1lt_pool.tile([128, NPCP], BF16, tag='h1LT')
        with tc.tile_pool(name='ptp', bufs=2, space='PSUM') as ptp:
            aggregate(table1, al1own, ST1, F1, H1, layer=1, emit=emit1)

        if stop == 'l1':
            _dbg_out(h1town_k[0], 128, min(G * 128, F2), BF16)
            h1lt_cm.__exit__(None, None, None)
            return

        # ---------------- Phase 4b: own-node ald2 (overlaps the AGs) -----
        with ExitStack() as S4:
            p4 = S4.enter_context(tc.tile_pool(name='p4sb', bufs=3))
            pp4 = S4.enter_context(
                tc.tile_pool(name='p4ps', bufs=4, space='PSUM'))
            for g0 in range(0, DT, G):
                ng = min(G, DT - g0)
                aldv4 = p4.tile([128, G, 1], F32, tag='aldv4')
                for t in range(ng):
                    c0 = (g0 + t) * 128
                    ps = pp4.tile([128, W2X], F32, space='PSUM')
                    nc.tensor.matmul(ps[:, :], lhsT=h1LT[:, c0:c0 + 128],
                                     rhs=W2sb[:, :], start=True, stop=True)
                    nc.scalar.copy(aldv4[:, t, :], ps[:, F2 + 1:F2 + 2])
                nc.scalar.dma_start(_rows(al2own, g0 * 128, ng, 0, 1),
                                    aldv4[:, 0:ng, :])
        h1lt_cm.__exit__(None, None, None)

        # ---------------- Phase 5: redundant GEMM2 for ALL nodes ---------
        with ExitStack() as S5:
            lp = S5.enter_context(tc.tile_pool(name='l5sb', bufs=3))
            p5 = S5.enter_context(tc.tile_pool(name='p5sb', bufs=3))
            pp5 = S5.enter_context(
                tc.tile_pool(name='p5ps', bufs=4, space='PSUM'))
            for cb in range(CORES):
                for g0 in range(0, DT, G):
                    ng = min(G, DT - g0)
                    lh = lp.tile([128, G * 128], BF16, tag='lh')
                    nc.sync.dma_start(
                        lh[:, 0:ng * 128],
                        h1T_k[g0 // G][cb * 128:(cb + 1) * 128, 0:ng * 128])
                    h2b = p5.tile([128, G, ST2], BF16, tag='h2b')
                    for t in range(ng):
                        ps = pp5.tile([128, W2X], F32, space='PSUM')
                        nc.tensor.matmul(ps[:, :],
                                         lhsT=lh[:, t * 128:(t + 1) * 128],
                                         rhs=W2sb[:, :], start=True, stop=True)
                        nc.scalar.copy(h2b[:, t, 0:F2], ps[:, 0:F2])
                    nc.scalar.dma_start(
                        _rows(h2full, cb * NPCP + g0 * 128, ng, 0, ST2),
                        h2b[:, 0:ng, :])

        if stop == 'p5':
            _dbg_out(h2full[NPCP:NPCP + NPC, :], NPC, min(ST2, F2), BF16)
            return

        # ---------------- Phase 6: layer-2 aggregation -------------------
        def emit2(t0, nd, Us, fp):
            c0 = t0 * 128
            ph = min(nd * 128, NPC - c0)
            Ub = fp.tile([128, CDn, F2 + 1], F32, tag='Ub2')
            for i, (U, sU) in enumerate(Us):
                nc.scalar.copy(Ub[:, i, 0:F2], U[:, :])
                nc.scalar.copy(Ub[:, i, F2:F2 + 1], sU[:, :])
            s_t = fp.tile([128, CDn, 1], F32, tag='s2')
            nc.vector.tensor_scalar(s_t[:, 0:nd, :],
                                    Ub[:, 0:nd, F2:F2 + 1], 1e-30, None,
                                    op0=OP.max)
            rec = fp.tile([128, CDn, 1], F32, tag='rec2')
            nc.vector.reciprocal(rec[:, 0:nd, :], s_t[:, 0:nd, :])
            hL = fp.tile([128, CDn, F2], F32, tag='hL2')
            nc.vector.tensor_tensor(
                hL[:, 0:nd, :], Ub[:, 0:nd, 0:F2],
                rec[:, 0:nd, :].to_broadcast([128, nd, F2]), op=OP.mult)
            nc.vector.tensor_tensor(hL[:, 0:nd, :], hL[:, 0:nd, :],
                                    _mid_bcast(sb['b2_rep'][:, :], nd),
                                    op=OP.add)
            scr2 = fp.tile([128, CDn, F2], F32, tag='scr2')
            ss = fp.tile([128, CDn, 1], F32, tag='ss')
            nc.vector.tensor_tensor(scr2[:, 0:nd, :], hL[:, 0:nd, :],
                                    hL[:, 0:nd, :], op=OP.mult)
            nc.vector.tensor_reduce(ss[:, 0:nd, :], scr2[:, 0:nd, :],
                                    axis=mybir.AxisListType.X, op=OP.add)
            nrm = fp.tile([128, CDn, 1], F32, tag='nrm')
            nc.scalar.sqrt(nrm[:, 0:nd, :], ss[:, 0:nd, :])
            nc.vector.tensor_scalar(nrm[:, 0:nd, :], nrm[:, 0:nd, :],
                                    1e-12, None, op0=OP.max)
            rc2 = fp.tile([128, CDn, 1], F32, tag='rc2')
            nc.vector.reciprocal(rc2[:, 0:nd, :], nrm[:, 0:nd, :])
            ot = fp.tile([128, CDn, F2], F32, tag='ot')
            # relu(hL / nrm) = max(hL, 0) * rc2  (rc2 > 0)
            nc.vector.scalar_tensor_tensor(
                ot[:, 0:nd, :], hL[:, 0:nd, :], 0.0,
                rc2[:, 0:nd, :].to_broadcast([128, nd, F2]),
                op0=OP.max, op1=OP.mult)
            if ph == nd * 128:
                nc.scalar.dma_start(_rows(d_out, c0, nd, 0, F2),
                                    ot[:, 0:nd, :])
            else:
                nfull = ph // 128
                if nfull:
                    nc.scalar.dma_start(_rows(d_out, c0, nfull, 0, F2),
                                        ot[:, 0:nfull, :])
                rem = ph - nfull * 128
                nc.scalar.dma_start(
                    d_out[c0 + nfull * 128:c0 + ph, :],
                    ot[:rem, nfull, :])

        aggregate(h2full, al2own, ST2, F2, 1, layer=2, emit=emit2)

    with tile.TileContext(nc) as tc:
        with ExitStack() as S:
            _body(tc, S)
    nc.compile()
    return nc


# ---------------------------------------------------------------------------
# PJRT runner: staged device inputs + donated output ping-pong
# ---------------------------------------------------------------------------

class Runner:
    def __init__(self, nc, n_cores):
        import jax
        from jax.sharding import Mesh, PartitionSpec, NamedSharding
        from jax.experimental.shard_map import shard_map
        from concourse.bass2jax import (_bass_exec_p, install_neuronx_cc_hook,
                                        partition_id_tensor)
        install_neuronx_cc_hook()
        self.jax = jax
        self.nc = nc
        self.n_cores = n_cores
        partition_name = (nc.partition_id_tensor.name
                          if nc.partition_id_tensor else None)
        in_names, out_names, out_avals = [], [], []
        for alloc in nc.m.functions[0].allocations:
            if not isinstance(alloc, mybir.MemoryLocationSet):
                continue
            name = alloc.memorylocations[0].name
            if alloc.kind == 'ExternalInput':
                if name != partition_name:
                    in_names.append(name)
            elif alloc.kind == 'ExternalOutput':
                out_names.append(name)
                out_avals.append(jax.core.ShapedArray(
                    tuple(alloc.tensor_shape), mybir.dt.np(alloc.dtype)))
        self.in_names, self.out_names, self.out_avals = (
            in_names, out_names, out_avals)
        n_params = len(in_names)
        n_outs = len(out_avals)
        all_in_names = list(in_names) + list(out_names)
        if partition_name is not None:
            all_in_names.append(partition_name)

        def _bd(*args):
            operands = list(args)
            if partition_name is not None:
                operands.append(partition_id_tensor())
            outs = _bass_exec_p.bind(
                *operands, out_avals=tuple(out_avals),
                in_names=tuple(all_in_names), out_names=tuple(out_names),
                lowering_input_output_aliases=(), sim_require_finite=True,
                sim_require_nnan=True, nc=nc)
            return tuple(outs)

        devs = jax.devices()[:n_cores]
        self.mesh = Mesh(np.asarray(devs), ('core',))
        self.sh = NamedSharding(self.mesh, PartitionSpec('core'))
        in_specs = (PartitionSpec('core'),) * (n_params + n_outs)
        out_specs = (PartitionSpec('core'),) * n_outs
        donate = tuple(range(n_params, n_params + n_outs))
        self.f = jax.jit(
            shard_map(_bd, mesh=self.mesh, in_specs=in_specs,
                      out_specs=out_specs, check_rep=False),
            donate_argnums=donate, keep_unused=True)
        import jax.numpy as jnp
        zshapes = [(n_cores * a.shape[0], *a.shape[1:]) for a in out_avals]
        zdt = [a.dtype for a in out_avals]
        self._zeros = jax.jit(
            lambda: tuple(jnp.zeros(s, d) for s, d in zip(zshapes, zdt)),
            out_shardings=(self.sh,) * n_outs)
        self._staged_key = None
        self._dev_in = None
        self._last_out = None

    def stage(self, in_maps, key):
        if self._staged_key == key and self._dev_in is not None:
            return
        per_core = [[np.asarray(m[nm]) for nm in self.in_names]
                    for m in in_maps]
        concat_in = [np.concatenate([per_core[cc][i]
                                     for cc in range(self.n_cores)], axis=0)
                     for i in range(len(self.in_names))]
        self._dev_in = [self.jax.device_put(a, self.sh) for a in concat_in]
        for a in self._dev_in:
            a.block_until_ready()
        self._staged_key = key
        self._last_out = None

    def exec_async(self):
        """One device execution; returns unfetched jax output arrays."""
        zo = self._last_out if self._last_out is not None else self._zeros()
        out = self.f(*self._dev_in, *zo)
        self._last_out = out
        return out

    def run(self):
        """Execute once and fetch outputs as a per-core list of dicts."""
        out = self.exec_async()
        for o in out:
            o.block_until_ready()
        res = []
        for cc in range(self.n_cores):
            d = {}
            for i, nm in enumerate(self.out_names):
                full = np.asarray(out[i])
                d[nm] = full.reshape(self.n_cores,
                                     *self.out_avals[i].shape)[cc]
            res.append(d)
        return res


# ---------------------------------------------------------------------------
# Entry point
# ---------------------------------------------------------------------------

_PROGRAMS = {}
_RUNNERS = {}
_PREP_CACHE = {}
_XCAST_CACHE = {}


def _crc(a):
    b = np.ascontiguousarray(a)
    return zlib.crc32(b.view(np.uint8).reshape(-1))


def _get_program(cfg, T_LO, T_HI, stop='full'):
    key = (cfg.N, cfg.E, cfg.CORES, T_LO, T_HI, cfg.CD, stop)
    if key not in _PROGRAMS:
        _PROGRAMS[key] = build_program(cfg, T_LO, T_HI, stop=stop)
    return _PROGRAMS[key]


def _get_runner(nc, n_cores):
    if id(nc) not in _RUNNERS:
        _RUNNERS[id(nc)] = Runner(nc, n_cores)
    return _RUNNERS[id(nc)]


def get_prepared(inputs, stop='full'):
    """Build/cache (cfg, runner) and stage inputs; shared with test.py."""
    x = np.asarray(inputs['x'])
    edge_index = np.asarray(inputs['edge_index'])
    n = x.shape[1]
    cfg = Cfg(N=n, E=edge_index.shape[1])

    ekey = (edge_index.shape, _crc(edge_index))
    if ekey not in _PREP_CACHE:
        _PREP_CACHE[ekey] = preprocess(edge_index, cfg)
    percore, T_LO, T_HI = _PREP_CACHE[ekey]

    xkey = (x.shape, _crc(x))
    if xkey not in _XCAST_CACHE:
        _XCAST_CACHE.clear()
        xf = np.asarray(x, np.float32).reshape(n, cfg.F0)
        _XCAST_CACHE[xkey] = np.ascontiguousarray(xf.T.astype(NP_BF16))
    xT_b16 = _XCAST_CACHE[xkey]

    nc = _get_program(cfg, T_LO, T_HI, stop=stop)
    runner = _get_runner(nc, cfg.CORES)
    wkey = tuple(_crc(np.asarray(inputs[k], np.float32))
                 for k in ('W1', 'a1_s', 'a1_d', 'b1',
                           'W2', 'a2_s', 'a2_d', 'b2'))
    skey = (ekey, xkey, wkey, stop)
    if runner._staged_key != skey:
        in_maps = make_in_maps(inputs, cfg, percore, T_LO, T_HI,
                               xT_b16=xT_b16)
        runner.stage(in_maps, skey)
    return cfg, runner


def kernel(**inputs) -> np.ndarray:
    cfg, runner = get_prepared(inputs)
    res = runner.run()
    out = np.concatenate([r['out'] for r in res], axis=0)
    return out.reshape(1, cfg.N, cfg.F2).astype(np.float32)
